# revision 18
# baseline (speedup 1.0000x reference)
"""Multi-head attention (B=4, S=2048, D=1024, H=16) on 8 trn2 NeuronCores.

Sharding: (batch, head-half) -> one core each, as in the baseline. v2
restructures the single-core schedule around the ScalarE exp bottleneck
(33.5M exps/core ~= 294us of ACT at 1 elem/cycle/lane):

  - Fat input DMAs: qt/kt/vt arrive as [128, 1024] half-row tiles (256KB
    transfers with 2KB partition lines) instead of [128,512]/[128,128]
    chunks, so the head is no longer DMA-starved (baseline ran its whole
    projection head at HAM K=4/8 half clock and started exp at t=152us).
  - Flat software pipeline: a scores+exp stream runs LEAD m-steps ahead of
    the PV stream in PE program order, so ACT stays saturated across group
    boundaries and through the group-0 v-projection chase.
  - Group order interleaves chunks 0/1 pair-major so the k/v/q projection
    chase spreads over 8 ACT windows instead of 4.
  - PSUM: scores 2x[128,1024] (4 banks), PV accumulators 2 banks (phase-1
    evacuation at group end), aux 2 banks shared by proj/Wo/norm-bcast.
  - reciprocal_approx_fast on the [1,512] denominator row before the PE
    broadcast (DVE norm chain ~131us -> ~50us).

Matmuls in bf16 (fp32 PSUM), exp on ACT with the 1/8 scale folded in,
denominators via the ones-column trick. Host sums the two partials per batch.
"""
import numpy as np

import concourse.bass as bass
import concourse.tile as tile
from concourse import mybir
from concourse.bass_utils import run_bass_kernel_spmd

F32 = mybir.dt.float32
F32R = mybir.dt.float32r
BF16 = mybir.dt.bfloat16
EXP = mybir.ActivationFunctionType.Exp

B, S, DM, H_TOT = 4, 2048, 1024, 16
F = 512          # features per core (8 heads x 64)
HD = 64          # head dim
NH = 8           # heads per core
NP = 4           # head pairs per core
KT = 16          # key tiles of 128
NQT = 4          # q chunks of 512
NG = NQT * NP    # 16 attention groups
SCALE = 0.125    # 1/sqrt(64)
N_CORES = 8
LEAD = 8         # scores/exp stream lead over PV stream, in m-steps

# pairs enter as late as possible (kf2 deadline sc96, kf3 sc144) and
# chunk-2 groups sit mid-stream, so the k/v/q projection chase spreads
# over ~10 ACT windows and both kernel halves balance
GROUP_ORDER = [
    (0, 0), (1, 0), (0, 1), (1, 1), (2, 0), (2, 1), (0, 2), (1, 2),
    (2, 2), (0, 3), (1, 3), (2, 3), (3, 0), (3, 1), (3, 2), (3, 3),
]

_WAIT_CAP = {"InstEventSemaphore": 2}


def _split_multiwaits(nc):
    """This walrus build accepts 1 sync-wait per instruction (2 on
    EventSemaphore); spread extras over same-engine NOPs placed before."""
    n_spill = 0
    for f in nc.m.functions:
        for bb in f.blocks:
            new = []
            changed = False
            for inst in bb.instructions:
                si = inst.sync_info
                cap = _WAIT_CAP.get(type(inst).__name__, 1)
                if si is not None and len(si.on_wait) > cap:
                    extra = list(si.on_wait[: len(si.on_wait) - cap])
                    del si.on_wait[: len(si.on_wait) - cap]
                    for w in extra:
                        n_spill += 1
                        nop = mybir.InstNoOp(name=f"I-wspill-{n_spill}-{inst.name}")
                        nop.engine = inst.engine
                        nop.sync_info = mybir.SyncInfo(on_wait=[w], on_update=[])
                        new.append(nop)
                    changed = True
                new.append(inst)
            if changed:
                bb.instructions[:] = new
    return n_spill


def build_program():
    nc = bass.Bass("TRN2", target_bir_lowering=False, debug=False, num_devices=1)

    d_qt = nc.dram_tensor("qt", [DM, S], BF16, kind="ExternalInput").ap()
    d_kt = nc.dram_tensor("kt", [DM, S], BF16, kind="ExternalInput").ap()
    d_vt = nc.dram_tensor("vt", [DM, S], BF16, kind="ExternalInput").ap()
    d_wq = nc.dram_tensor("wq", [DM, F], BF16, kind="ExternalInput").ap()
    d_wk = nc.dram_tensor("wk", [DM, F], BF16, kind="ExternalInput").ap()
    d_wv = nc.dram_tensor("wv", [DM, F], BF16, kind="ExternalInput").ap()
    d_wo = nc.dram_tensor("wo", [F, DM], BF16, kind="ExternalInput").ap()
    d_bq = nc.dram_tensor("bq", [F], F32, kind="ExternalInput").ap()
    d_bk = nc.dram_tensor("bk", [F], F32, kind="ExternalInput").ap()
    d_bv = nc.dram_tensor("bv", [F], F32R, kind="ExternalInput").ap()
    d_ones = nc.dram_tensor("ones", [1, 128], F32R, kind="ExternalInput").ap()
    d_part = nc.dram_tensor("part", [S, DM], F32, kind="ExternalOutput").ap()

    with tile.TileContext(nc) as tc:
        with (
            tc.tile_pool(name="wpool", bufs=1) as wpool,
            tc.tile_pool(name="ktp", bufs=1) as ktp,
            tc.tile_pool(name="qtp", bufs=2) as qtp,
            tc.tile_pool(name="vtp", bufs=3) as vtp,
            tc.tile_pool(name="big", bufs=1) as big,
            tc.tile_pool(name="exch", bufs=9) as exch,
            tc.tile_pool(name="outst", bufs=2) as outst,
            tc.tile_pool(name="ocp", bufs=2) as ocp,
            tc.tile_pool(name="dnp", bufs=2) as dnp,
            tc.tile_pool(name="ps_sc", bufs=2, space="PSUM") as ps_sc,
            tc.tile_pool(name="ps_po", bufs=2, space="PSUM") as ps_po,
            tc.tile_pool(name="ps_aux", bufs=2, space="PSUM") as ps_aux,
        ):
            # ---- resident tiles
            # weights in single block-major tiles: w[:, 512*m + c] = W[128*m + p, c]
            wq_sb = wpool.tile([128, 8 * F], BF16, tag="wq", name="wq")
            wk_sb = wpool.tile([128, 8 * F], BF16, tag="wk", name="wk")
            wv_sb = wpool.tile([128, 8 * F], BF16, tag="wv", name="wv")
            wo_sb = wpool.tile([128, 4 * DM], BF16, tag="wo", name="wo")
            qT_sb = [big.tile([128, S], BF16, tag=f"qT{f}", name=f"qT{f}") for f in range(4)]
            kT_sb = [big.tile([128, S], BF16, tag=f"kT{f}", name=f"kT{f}") for f in range(4)]
            oT_sb = [big.tile([128, S], BF16, tag=f"oT{f}", name=f"oT{f}") for f in range(4)]
            v_sb = [big.tile([128, NH * (HD + 1)], BF16, tag=f"v{t}", name=f"v{t}") for t in range(KT)]
            bq_sb = wpool.tile([128, 4], F32, tag="bq")
            bk_sb = wpool.tile([128, 4], F32, tag="bk")
            bv_sb = wpool.tile([1, F], F32R, tag="bv")
            ones_sb = wpool.tile([1, 128], F32R, tag="ones")
            bvbc_sb = wpool.tile([128, F], F32, tag="bvbc")
            warm_sb = wpool.tile([1, 16], F32, tag="warm")

            # ---- DMA head, ordered for earliest scores start
            nc.sync.dma_start(bq_sb[:], d_bq.rearrange("(f p) -> p f", p=128))
            nc.sync.dma_start(bk_sb[:], d_bk.rearrange("(f p) -> p f", p=128))
            nc.sync.dma_start(bv_sb[:], d_bv.rearrange("(a f) -> a f", a=1))
            nc.sync.dma_start(ones_sb[:], d_ones[:])
            nc.sync.dma_start(
                wk_sb[:].rearrange("p (b f) -> p b f", b=8),
                d_wk.rearrange("(b p) f -> p b f", p=128))

            kt_t = {}
            qt_t = {}
            vt_t = {}

            def load_kt(h):
                for m in range(8):
                    t = ktp.tile([128, 1024], BF16, tag=f"kt{h}{m}", name=f"kt{h}{m}")
                    nc.sync.dma_start(
                        t[:], d_kt[128 * m:128 * (m + 1), 1024 * h:1024 * (h + 1)])
                    kt_t[(h, m)] = t

            def load_qt(n):
                # one DMA per 512-token chunk, block-major [128, 4096]
                t = qtp.tile([128, 4096], BF16, tag="qt", name=f"qt{n}")
                nc.sync.dma_start(
                    t[:].rearrange("p (b c) -> p b c", b=8),
                    d_qt[:, 512 * n:512 * (n + 1)].rearrange(
                        "(b p) c -> p b c", p=128))
                qt_t[n] = t

            def load_vt(q):
                # one DMA per 512-token quarter, block-major [128, 4096];
                # ring of 3 so quarter 3 overlaps pieces of quarters 1-2
                t = vtp.tile([128, 4096], BF16, tag="vt", name=f"vt{q}")
                nc.sync.dma_start(
                    t[:].rearrange("p (b c) -> p b c", b=8),
                    d_vt[:, 512 * q:512 * (q + 1)].rearrange(
                        "(b p) c -> p b c", p=128))
                vt_t[q] = t

            # DMA order tuned so scores (wk+ktA+wq+qtc0) unblock earliest,
            # then ktB (scores m>=8), then the v-projection inputs
            load_kt(0)
            nc.sync.dma_start(
                wq_sb[:].rearrange("p (b f) -> p b f", b=8),
                d_wq.rearrange("(b p) f -> p b f", p=128))
            load_qt(0)
            load_kt(1)
            nc.sync.dma_start(
                wv_sb[:].rearrange("p (b f) -> p b f", b=8),
                d_wv.rearrange("(b p) f -> p b f", p=128))
            load_vt(0)
            load_vt(1)
            load_vt(2)
            load_qt(1)
            nc.sync.dma_start(
                wo_sb[:].rearrange("p (b f) -> p b f", b=4),
                d_wo.rearrange("(b p) f -> p b f", p=128))

            # warm the ACT exp table during the DMA head
            nc.vector.memset(warm_sb[:], 0.0)
            with nc.allow_low_precision(reason="act table warm dummy"):
                nc.scalar.activation(warm_sb[:], warm_sb[:], EXP, scale=1.0)

            # bv broadcast over partitions via K=1 matmul, repeated as PE
            # warm-up filler so HAM is at K=8/8 when the kt DMAs land
            psbv = ps_aux.tile([128, 512], F32, tag="aux", name="psbv")
            for _ in range(20):
                nc.tensor.matmul(psbv[:], ones_sb[0:1, :], bv_sb[0:1, :])
            nc.vector.tensor_copy(bvbc_sb[:], psbv[:])

            # ---- projection pieces reading resident/staged input tiles
            def qk_piece(w_sb, rhs_of_m, bias_sb, dst_sb, n, f):
                accp = ps_aux.tile([128, 512], F32, tag="aux", name="accp")
                for m in range(8):
                    nc.tensor.matmul(
                        accp[:],
                        w_sb[:, 512 * m + 128 * f:512 * m + 128 * (f + 1)],
                        rhs_of_m(m),
                        start=(m == 0),
                        stop=(m == 7),
                    )
                with nc.allow_low_precision(reason="bf16 qT/kT store"):
                    nc.vector.tensor_scalar_add(
                        dst_sb[f][:, 512 * n:512 * (n + 1)],
                        accp[:],
                        bias_sb[:, f:f + 1],
                    )

            def k_piece(n, f):
                h, c = n // 2, n % 2
                qk_piece(
                    wk_sb,
                    lambda m: kt_t[(h, m)][:, 512 * c:512 * (c + 1)],
                    bk_sb, kT_sb, n, f)

            def q_piece(n, f):
                qk_piece(
                    wq_sb, lambda m: qt_t[n][:, 512 * m:512 * (m + 1)],
                    bq_sb, qT_sb, n, f)

            def v_piece(t):
                acc = ps_aux.tile([128, 512], F32, tag="aux", name="accv")
                q, c = t // 4, t % 4
                for m in range(8):
                    nc.tensor.matmul(
                        acc[:],
                        vt_t[q][:, 512 * m + 128 * c:512 * m + 128 * (c + 1)],
                        wv_sb[:, 512 * m:512 * (m + 1)],
                        start=(m == 0),
                        stop=(m == 7),
                    )
                v3 = v_sb[t][:].rearrange("p (h e) -> p h e", e=HD + 1)
                nc.vector.memset(v3[:, :, HD:HD + 1], 1.0)
                nc.vector.tensor_add(
                    v3[:, :, 0:HD],
                    acc[:].rearrange("p (h e) -> p h e", e=HD),
                    bvbc_sb[:].rearrange("p (h e) -> p h e", e=HD),
                )

            def wo_unit(tt, j):
                tsl = slice(128 * tt, 128 * (tt + 1))
                pw = ps_aux.tile([128, 512], F32, tag="aux", name="pw")
                for f in range(4):
                    nc.tensor.matmul(
                        pw[:], oT_sb[f][:, tsl],
                        wo_sb[:, 1024 * f + 512 * j:1024 * f + 512 * (j + 1)],
                        start=(f == 0), stop=(f == 3),
                    )
                ost = outst.tile([128, 512], F32, tag="outst")
                nc.vector.tensor_copy(ost[:], pw[:])
                nc.sync.dma_start(d_part[tsl, 512 * j:512 * (j + 1)], ost[:])

            def phase1(po):
                oc = ocp.tile([65, 512], F32, tag="oc", name="oc")
                nc.vector.tensor_copy(oc[:], po[0:65, :])
                return oc

            def make_norm(p, qsl, i, oc):
                # split in two slots so the 3.4us DVE reciprocal never sits
                # in front of a PE instruction in the FIFO: `pre` issues the
                # reciprocal (DVE-only), `post` a few m-steps later does the
                # PE broadcast + multiply, by which time the recip is done
                state = {}

                def pre():
                    dnr = dnp.tile([1, 512], F32, tag="dn", name="dnr", bufs=1)
                    with nc.allow_low_precision(reason="f32 softmax denom recip"):
                        nc.vector.reciprocal(dnr[:], oc[64:65, :])
                    dnr2 = dnp.tile([1, 512], F32R, tag="dn2", name="dnr2")
                    nc.vector.tensor_copy(dnr2[:], dnr[:])
                    state["dnr2"] = dnr2

                def post():
                    r0 = 64 * i
                    pb = ps_aux.tile([128, 512], F32, tag="aux", name="pb")
                    nc.tensor.matmul(
                        pb[0:64, :], ones_sb[0:1, 0:64], state["dnr2"][0:1, :]
                    )
                    with nc.allow_low_precision(reason="bf16 normalized out"):
                        nc.vector.tensor_mul(
                            oT_sb[p][r0:r0 + 64, qsl], oc[0:64, :], pb[0:64, :]
                        )
                return pre, post

            # ---- head PE work (minimal prefix before the pipeline):
            # k-pair0 pieces for key chunks 0,1; kf1 starts filling the
            # qtA DMA wait; q piece (0,0). kf0 n=2,3 go in as sc-step
            # fillers once ktB lands.
            k_piece(0, 0)
            q_piece(0, 0)

            # ---- sc-side fillers, one piece per assigned global sc step
            # (spread EDF-style: bursts of >1.7us between consecutive score
            # emissions starve ACT, so each piece gets its own slot well
            # before its deadline sc(group-first-use, 4n))
            sc_pre = {}

            def at(step, fn):
                sc_pre.setdefault(step, []).append(fn)

            at(2, lambda: k_piece(1, 0))       # scores g0 m4
            at(4, lambda: k_piece(2, 0))       # scores g0 m8 (ktB-paced)
            at(6, lambda: k_piece(3, 0))
            at(8, lambda: k_piece(0, 1))       # kT[1] for g2 (sc32)
            at(10, lambda: k_piece(1, 1))
            at(12, lambda: q_piece(1, 0))      # scores g1 (sc16)
            at(17, lambda: q_piece(0, 1))      # g2 (32)
            at(20, lambda: k_piece(2, 1))      # g2 m8 (40)
            at(23, lambda: k_piece(3, 1))      # g2 m12 (44)
            at(26, lambda: q_piece(1, 1))      # g3 (48)
            at(30, lambda: q_piece(0, 2))      # early: frees qt c0
            at(35, lambda: q_piece(0, 3))      # early: frees qt c0
            at(38, lambda: load_qt(2))
            at(44, lambda: q_piece(2, 0))      # g4 (64)
            at(48, lambda: q_piece(2, 1))      # g5 (80)
            at(52, lambda: q_piece(1, 2))      # g7 (112); early frees qt c1
            at(58, lambda: q_piece(1, 3))      # g10 (160); early frees qt c1
            at(62, lambda: q_piece(2, 2))      # g8 (128)
            at(65, lambda: load_qt(3))
            at(70, lambda: k_piece(0, 2))      # kT[2] for g6 (sc96)
            at(75, lambda: k_piece(1, 2))
            at(80, lambda: k_piece(2, 2))      # g6 m8 (104)
            at(85, lambda: k_piece(3, 2))      # g6 m12 (108)
            at(100, lambda: q_piece(2, 3))     # g11 (176)
            at(110, lambda: k_piece(0, 3))     # kT[3] for g9 (sc144)
            at(117, lambda: k_piece(1, 3))
            at(124, lambda: k_piece(2, 3))     # g9 m8 (152)
            at(131, lambda: k_piece(3, 3))     # g9 m12 (156)
            at(150, lambda: q_piece(3, 0))     # g12 (192)
            at(170, lambda: q_piece(3, 1))     # g13 (208)
            at(190, lambda: q_piece(3, 2))     # g14 (224)
            at(210, lambda: q_piece(3, 3))     # g15 (240)

            # pv-side fillers at (group, m) slots: Wo units for chunk c
            # once all its pairs' norms have landed
            pv_fill = {}
            # Wo: FIFO of 32 units, 4 per group from g10 on (chunk c is
            # normed by group {g10,g11,g12}[c]-m10); chunk 3 in the tail
            wo_q = [(4 * c + (2 * gi + u // 2), u % 2)
                    for c in range(3) for gi in range(2) for u in range(4)]
            for g in range(10, NG):
                for s in (11, 13, 14, 15):
                    if wo_q:
                        tt, j = wo_q.pop(0)
                        pv_fill.setdefault((g, s), []).append(
                            lambda tt=tt, j=j: wo_unit(tt, j))
            pv_fill.setdefault((0, 4), []).append(lambda: load_vt(3))

            # ---- flat pipeline: scores+exp stream LEAD steps ahead of PV
            def sc_emit(step):
                g, m = divmod(step, KT)
                n, p = GROUP_ORDER[g]
                qsl = slice(512 * n, 512 * (n + 1))
                scp = ps_sc.tile([128, 1024], F32, tag="sc", name="scp")
                ksl = slice(128 * m, 128 * (m + 1))
                nc.tensor.matmul(
                    scp[:, 0:512], kT_sb[p][0:64, ksl], qT_sb[p][0:64, qsl],
                    tile_position=(0, 0),
                )
                nc.tensor.matmul(
                    scp[:, 512:1024], kT_sb[p][64:128, ksl],
                    qT_sb[p][64:128, qsl], tile_position=(64, 0),
                )
                ex = exch.tile([128, 1024], BF16, tag="ex", name="ex")
                nc.scalar.activation(ex[:], scp[:], EXP, scale=SCALE)
                return ex

            ex_tiles = {}
            sc_done = 0
            poA = poB = None
            for step in range(NG * KT):
                g, m = divmod(step, KT)
                n, p = GROUP_ORDER[g]
                qsl = slice(512 * n, 512 * (n + 1))
                if m == 0:
                    poA = ps_po.tile([128, 512], F32, tag="po", name="poA")
                    poB = ps_po.tile([128, 512], F32, tag="po", name="poB")
                while sc_done < min(NG * KT, step + LEAD + 1):
                    for fn in sc_pre.get(sc_done, []):
                        fn()
                    ex_tiles[sc_done] = sc_emit(sc_done)
                    sc_done += 1
                if g == 0:
                    v_piece(m)
                ex = ex_tiles.pop(step)
                nc.tensor.matmul(
                    poA[0:65, :], v_sb[m][:, 130 * p:130 * p + 65],
                    ex[:, 0:512], start=(m == 0), stop=(m == KT - 1),
                )
                nc.tensor.matmul(
                    poB[0:65, :], v_sb[m][:, 130 * p + 65:130 * p + 130],
                    ex[:, 512:1024], start=(m == 0), stop=(m == KT - 1),
                )
                for fn in pv_fill.get((g, m), []):
                    fn()
                if m == KT - 1:
                    ocA, ocB = phase1(poA), phase1(poB)
                    nA = make_norm(p, qsl, 0, ocA)
                    nB = make_norm(p, qsl, 1, ocB)
                    if g + 1 < NG:
                        pv_fill.setdefault((g + 1, 3), []).append(nA[0])
                        pv_fill.setdefault((g + 1, 5), []).append(nB[0])
                        pv_fill.setdefault((g + 1, 8), []).append(nA[1])
                        pv_fill.setdefault((g + 1, 10), []).append(nB[1])
                    else:
                        nA[0]()
                        nB[0]()
                        nA[1]()
                        nB[1]()

            # ---- tail: chunk-3 Wo
            for tt in range(12, 16):
                for j in range(2):
                    wo_unit(tt, j)

    _split_multiwaits(nc)
    return nc


_PROGRAM = None


def _get_program():
    global _PROGRAM
    if _PROGRAM is None:
        _PROGRAM = build_program()
    return _PROGRAM


def make_in_maps(Q, K, V, Wq, bq, Wk, bk, Wv, bv, Wo, bo):
    import ml_dtypes
    bf = lambda x: np.asarray(x, dtype=np.float32).astype(ml_dtypes.bfloat16)
    f32 = lambda x: np.asarray(x, dtype=np.float32)
    Q, K, V = bf(Q), bf(K), bf(V)
    Wq, Wk, Wv, Wo = bf(Wq), bf(Wk), bf(Wv), bf(Wo)
    bq, bk, bv = f32(bq), f32(bk), f32(bv)
    ones = np.ones((1, 128), np.float32)
    in_maps = []
    for c in range(N_CORES):
        b, hh = c // 2, c % 2
        fs = slice(F * hh, F * (hh + 1))
        in_maps.append({
            "qt": np.ascontiguousarray(Q[b].T),
            "kt": np.ascontiguousarray(K[b].T),
            "vt": np.ascontiguousarray(V[b].T),
            "wq": np.ascontiguousarray(Wq[:, fs]),
            "wk": np.ascontiguousarray(Wk[:, fs]),
            "wv": np.ascontiguousarray(Wv[:, fs]),
            "wo": np.ascontiguousarray(Wo[fs, :]),
            "bq": np.ascontiguousarray(bq[fs]),
            "bk": np.ascontiguousarray(bk[fs]),
            "bv": np.ascontiguousarray(bv[fs]),
            "ones": ones,
        })
    return in_maps


def kernel(Q, K, V, Wq, bq, Wk, bk, Wv, bv, Wo, bo, _trace=False, _trace_kwargs=None):
    nc = _get_program()
    in_maps = make_in_maps(Q, K, V, Wq, bq, Wk, bk, Wv, bv, Wo, bo)
    res = run_bass_kernel_spmd(
        nc, in_maps, core_ids=list(range(N_CORES)),
        trace=_trace, **(_trace_kwargs or {}),
    )
    parts = [r["part"] for r in res.results]
    out = np.stack([parts[2 * b] + parts[2 * b + 1] for b in range(B)])
    out += np.asarray(bo, dtype=np.float32)[None, None, :]
    if _trace:
        return out, res
    return out
